# revision 1
# baseline (speedup 1.0000x reference)
"""BiLSTM-CRF sequence tagging loss on 8 Trainium2 NeuronCores.

Data-parallel: batch 128 sharded 16/core across 8 cores; each core runs the
full model (embedding gather, 2 BiLSTM layers, FC, CRF forward algorithm)
on its own shard with zero cross-core communication. Host sums the 8
per-core partial losses.

Key device-side tricks:
  - sigmoid-free LSTM cell: i,f,o weight rows pre-scaled by 0.5 so a single
    tanh activation covers all four gates (sigmoid(x) = 0.5*tanh(x/2)+0.5),
    with doubled cell/hidden state (C^=2c, h^=2h) so the whole pointwise
    update is exactly 4 fused scalar_tensor_tensor DVE ops per step.
  - xg (input projection) folded into PSUM via an identity matmul so the
    gate pre-activation needs no extra elementwise add.
  - CRF partition function in the exp domain (scaled forward algorithm):
    per step one 20x20 stationary matmul + one fused DVE multiply; 1/20
    folded into exp(emissions); periodic data-dependent rescale via a
    ones-matmul partition reduction, log-corrections accumulated on device.
"""

import numpy as np

V, E, H, C = 50000, 300, 256, 20
B, T_FULL = 128, 512
N_CORES = 8
B_LOC = B // N_CORES  # 16
G4 = 4 * H  # 1024
RESCALE_EVERY = 24

_COMPILED = {}


def _build(T, debug=False, phases='ABCDEF', reps=1):
    import concourse.bass as bass
    import concourse.mybir as mybir
    import concourse.tile as tile
    from concourse import bacc
    from concourse.masks import make_identity
    from contextlib import ExitStack

    f32 = mybir.dt.float32
    bf16 = mybir.dt.bfloat16
    i32 = mybir.dt.int32
    AF = mybir.ActivationFunctionType
    OP = mybir.AluOpType

    NTOK = T * B_LOC            # tokens per core
    NM = NTOK // 128            # 128-token m-tiles
    E_CH = [(0, 128), (128, 128), (256, 44)]   # E=300 chunks

    nc = bacc.Bacc("TRN2", debug=False, num_devices=N_CORES)

    def din(name, shape, dt=f32):
        return nc.dram_tensor(name, shape, dt, kind="ExternalInput").ap()

    ids_d = din("ids", (128, NM), i32)
    emb_d = din("emb", (V, E))
    w0i_d = din("w0i", (2, E, G4))
    w0h_d = din("w0h", (2, H, G4))
    b0_d = din("b0r", (128, 2, G4))
    w1i_d = din("w1i", (2, 2 * H, G4))
    w1h_d = din("w1h", (2, H, G4))
    b1_d = din("b1r", (128, 2, G4))
    fct_d = din("fcT", (2 * H, C))
    fcb_d = din("fcbr", (128, C))
    mask_d = din("maskE", (128, NM * C))
    sel_d = din("selm", (128, B_LOC))
    pm_d = din("Pm", (C, C))
    est_d = din("estart", (C, 1))
    een_d = din("eend", (C, 1))
    chain_d = din("chain", (1, 1))

    s_out = nc.dram_tensor("S_out", (1, B_LOC), f32, kind="ExternalOutput").ap()
    ne_out = nc.dram_tensor("numE_out", (1, B_LOC), f32, kind="ExternalOutput").ap()
    la_out = nc.dram_tensor("logacc_out", (1, B_LOC), f32, kind="ExternalOutput").ap()

    # DRAM scratch
    xg0_d = nc.dram_tensor("xg0", (2, NTOK, G4), f32).ap()
    xg1_d = nc.dram_tensor("xg1", (2, NTOK, G4), f32).ap()
    dbg_kind = "ExternalOutput" if debug else "Internal"
    hc0_d = nc.dram_tensor("hcat0", (T, B_LOC, 2 * H), f32, kind=dbg_kind).ap()
    hc1_d = nc.dram_tensor("hcat1", (T, B_LOC, 2 * H), f32, kind=dbg_kind).ap()

    with tile.TileContext(nc) as tc, ExitStack() as top:
        cp = top.enter_context(tc.tile_pool(name="const", bufs=1))

        ident = cp.tile([128, 128], f32)
        make_identity(nc, ident[:])
        id16f = cp.tile([16, 16], f32)
        make_identity(nc, id16f[:])
        id16b = cp.tile([16, 16], bf16)
        nc.vector.tensor_copy(id16b[:], id16f[:])
        # identity replicated at partition offsets 0 and 32 (for dir-1 transposes)
        id64f = cp.tile([64, 16], f32)
        nc.gpsimd.memset(id64f[:], 0.0)
        make_identity(nc, id64f[0:16, :], nomemset=True)
        make_identity(nc, id64f[32:48, :], nomemset=True)

        ids_sb = cp.tile([128, NM], i32)
        nc.sync.dma_start(ids_sb[:], ids_d[:])

        # weight staging is per-phase (SBUF freed between phases)
        def load_w(pool, dram, rows, name):
            out = []
            for d in range(2):
                chs = []
                r0 = 0
                while r0 < rows:
                    ck = min(128, rows - r0)
                    t = pool.tile([ck, G4], f32, tag=f"{name}{d}_{r0}", name=f"{name}{d}_{r0}")
                    nc.sync.dma_start(t[:], dram[d, r0 : r0 + ck, :])
                    chs.append((t, ck))
                    r0 += ck
                out.append(chs)
            return out
        fct_sb = []
        for k in range(4):
            t = cp.tile([128, C], f32, tag=f"fct{k}")
            nc.sync.dma_start(t[:], fct_d[k * 128 : (k + 1) * 128, :])
            fct_sb.append(t)
        fcb_sb = cp.tile([128, C], f32)
        nc.sync.dma_start(fcb_sb[:], fcb_d[:])
        mask_sb = cp.tile([128, NM * C], f32)
        nc.sync.dma_start(mask_sb[:], mask_d[:])
        sel_sb = cp.tile([128, B_LOC], f32)
        nc.sync.dma_start(sel_sb[:], sel_d[:])
        pm_sb = cp.tile([C, C], f32)
        nc.sync.dma_start(pm_sb[:], pm_d[:])
        est_sb = cp.tile([C, 1], f32)
        nc.sync.dma_start(est_sb[:], est_d[:])
        een_sb = cp.tile([C, 1], f32)
        nc.sync.dma_start(een_sb[:], een_d[:])
        ones20 = cp.tile([C, C], f32)
        nc.vector.memset(ones20[:], 1.0)
        chain_sb = cp.tile([1, 1], f32)
        nc.sync.dma_start(chain_sb[:], chain_d[:])

        def whole_model():
            # ---------------- Phase A: gather + transpose + L0 input proj ----------
            if 'A' in phases:
              with ExitStack() as es:
                sp = es.enter_context(tc.tile_pool(name="pA", bufs=3))
                wp = es.enter_context(tc.tile_pool(name="pAw", bufs=1))
                pp = es.enter_context(tc.tile_pool(name="pAp", bufs=2, space="PSUM"))
                w0i_sb = load_w(wp, w0i_d, E, "w0i")
                b0_sb = wp.tile([128, 2 * G4], f32)
                nc.sync.dma_start(b0_sb[:], b0_d[:].rearrange("p a b -> p (a b)"))
                for m in range(NM):
                    xm = sp.tile([128, E], f32, tag="xm")
                    nc.gpsimd.indirect_dma_start(
                        out=xm[:],
                        out_offset=None,
                        in_=emb_d[:],
                        in_offset=bass.IndirectOffsetOnAxis(ap=ids_sb[:, m : m + 1], axis=0),
                    )
                    xTm = []
                    for (r0, ck) in E_CH:
                        tp = pp.tile([128, 128], f32, tag="tp")
                        nc.tensor.transpose(out=tp[:ck, :], in_=xm[:, r0 : r0 + ck], identity=ident[:])
                        xt = sp.tile([128, 128], f32, tag=f"xt{r0}")
                        nc.scalar.copy(out=xt[:ck, :], in_=tp[:ck, :])
                        xTm.append((xt, ck))
                    for d in range(2):
                        ps = pp.tile([128, G4], f32, tag="psxg")
                        for ki, (xt, ck) in enumerate(xTm):
                            for nb in range(2):
                                nc.tensor.matmul(
                                    ps[:, nb * 512 : (nb + 1) * 512],
                                    lhsT=xt[:ck, :],
                                    rhs=w0i_sb[d][ki][0][:ck, nb * 512 : (nb + 1) * 512],
                                    start=(ki == 0),
                                    stop=(ki == 2),
                                )
                        ev = sp.tile([128, G4], f32, tag="ev")
                        nc.vector.scalar_tensor_tensor(
                            out=ev[:], in0=ps[:], scalar=0.0,
                            in1=b0_sb[:, d * G4 : (d + 1) * G4],
                            op0=OP.add, op1=OP.add,
                        )
                        nc.sync.dma_start(xg0_d[d, m * 128 : (m + 1) * 128, :], ev[:])

            # ---------------- LSTM recurrence (shared for L0/L1) -------------------
            # Two fully independent per-direction chains so their ~7-hop
            # cross-engine dependency latencies overlap.
            def recurrence(xg_d, wh_d, hout_d):
                with ExitStack() as es:
                    rp = es.enter_context(tc.tile_pool(name="rec", bufs=1))
                    sp = es.enter_context(tc.tile_pool(name="recw", bufs=4))
                    gp = es.enter_context(tc.tile_pool(name="recG", bufs=1, space="PSUM"))
                    tp = es.enter_context(tc.tile_pool(name="recT", bufs=4, space="PSUM"))
                    wh_sb = load_w(rp, wh_d, H, "wh")

                    G = [gp.tile([16, G4], f32, tag=f"G{d}", name=f"G{d}") for d in range(2)]
                    Cst = [rp.tile([16, H], f32, tag=f"C{d}", name=f"C{d}") for d in range(2)]
                    hT = [[rp.tile([128, 16], f32, tag=f"hT{d}_{k}", name=f"hT{d}_{k}") for k in range(2)] for d in range(2)]
                    for d in range(2):
                        nc.vector.memset(Cst[d][:], 0.0)
                        for k in range(2):
                            nc.vector.memset(hT[d][k][:], 0.0)

                    def step(d, t):
                        tt = t if d == 0 else T - 1 - t
                        g = G[d]
                        xgt = sp.tile([16, G4], f32, tag=f"xgt{d}", name=f"xgt{d}")
                        nc.sync.dma_start(xgt[:], xg_d[d, tt * 16 : (tt + 1) * 16, :])
                        for nb in range(2):
                            nc.tensor.matmul(
                                g[:, nb * 512 : (nb + 1) * 512],
                                lhsT=id16f[:],
                                rhs=xgt[:, nb * 512 : (nb + 1) * 512],
                                start=True, stop=False,
                            )
                        for k in range(2):
                            for nb in range(2):
                                nc.tensor.matmul(
                                    g[:, nb * 512 : (nb + 1) * 512],
                                    lhsT=hT[d][k][:],
                                    rhs=wh_sb[d][k][0][:, nb * 512 : (nb + 1) * 512],
                                    start=False, stop=(k == 1),
                                )
                        Tall = sp.tile([16, G4], f32, tag=f"Tall{d}", name=f"Tall{d}")
                        nc.scalar.activation(Tall[:], g[:], AF.Tanh)
                        A = sp.tile([16, H], f32, tag=f"A{d}", name=f"A{d}")
                        nc.vector.scalar_tensor_tensor(
                            out=A[:], in0=Tall[:, 256:512], scalar=1.0, in1=Cst[d][:],
                            op0=OP.add, op1=OP.mult)
                        Bv = sp.tile([16, H], f32, tag=f"Bv{d}", name=f"Bv{d}")
                        nc.vector.scalar_tensor_tensor(
                            out=Bv[:], in0=Tall[:, 0:256], scalar=1.0, in1=Tall[:, 512:768],
                            op0=OP.add, op1=OP.mult)
                        nc.vector.scalar_tensor_tensor(
                            out=Cst[d][:], in0=A[:], scalar=0.5, in1=Bv[:],
                            op0=OP.mult, op1=OP.add)
                        TC = sp.tile([16, H], f32, tag=f"TC{d}", name=f"TC{d}")
                        nc.scalar.activation(TC[:], Cst[d][:], AF.Tanh, scale=0.5)
                        Hh = sp.tile([16, H], f32, tag=f"Hh{d}", name=f"Hh{d}")
                        nc.vector.scalar_tensor_tensor(
                            out=Hh[:], in0=Tall[:, 768:1024], scalar=1.0, in1=TC[:],
                            op0=OP.add, op1=OP.mult)
                        for k in range(2):
                            tps = tp.tile([128, 16], f32, tag="tps", name="tps")
                            nc.tensor.transpose(
                                out=tps[:], in_=Hh[:, k * 128 : (k + 1) * 128],
                                identity=id16f[:])
                            nc.scalar.copy(out=hT[d][k][:], in_=tps[:])
                        nc.sync.dma_start(hout_d[tt, :, d * H : (d + 1) * H], Hh[:])

                    for t in range(T):
                        step(0, t)
                        step(1, t)

            if 'B' in phases:
                recurrence(xg0_d, w0h_d, hc0_d)

            # ---------------- Phase C: L1 input proj from hcat0 --------------------
            def proj_l1(hin_d, xgo_d):
                with ExitStack() as es:
                    sp = es.enter_context(tc.tile_pool(name="pC", bufs=3))
                    wp = es.enter_context(tc.tile_pool(name="pCw", bufs=1))
                    pp = es.enter_context(tc.tile_pool(name="pCp", bufs=2, space="PSUM"))
                    w1i_sb = load_w(wp, w1i_d, 2 * H, "w1i")
                    b1_sb = wp.tile([128, 2 * G4], f32)
                    nc.sync.dma_start(b1_sb[:], b1_d[:].rearrange("p a b -> p (a b)"))
                    for m in range(NM):
                        hm = sp.tile([128, 2 * H], f32, tag="hm")
                        nc.sync.dma_start(
                            hm[:], hin_d[:].rearrange("t b h -> (t b) h")[m * 128 : (m + 1) * 128, :])
                        hTm = []
                        for k in range(4):
                            tps = pp.tile([128, 128], f32, tag="tp")
                            nc.tensor.transpose(out=tps[:], in_=hm[:, k * 128 : (k + 1) * 128], identity=ident[:])
                            ht = sp.tile([128, 128], f32, tag=f"ht{k}")
                            nc.scalar.copy(out=ht[:], in_=tps[:])
                            hTm.append(ht)
                        for d in range(2):
                            ps = pp.tile([128, G4], f32, tag="psxg")
                            for k in range(4):
                                for nb in range(2):
                                    nc.tensor.matmul(
                                        ps[:, nb * 512 : (nb + 1) * 512],
                                        lhsT=hTm[k][:],
                                        rhs=w1i_sb[d][k][0][:, nb * 512 : (nb + 1) * 512],
                                        start=(k == 0), stop=(k == 3),
                                    )
                            ev = sp.tile([128, G4], f32, tag="ev")
                            nc.vector.scalar_tensor_tensor(
                                out=ev[:], in0=ps[:], scalar=0.0,
                                in1=b1_sb[:, d * G4 : (d + 1) * G4],
                                op0=OP.add, op1=OP.add)
                            nc.sync.dma_start(xgo_d[d, m * 128 : (m + 1) * 128, :], ev[:])

            if 'C' in phases:
                proj_l1(hc0_d, xg1_d)
            if 'D' in phases:
                recurrence(xg1_d, w1h_d, hc1_d)

            # ---------------- Phase E: FC -> emissions, numE, exp-emissions --------
            ET = cp.tile([C, NTOK], f32)          # exp(e)/C, transposed [C, tok]
            accT = cp.tile([128, NM], f32)
            nlnC = cp.tile([128, 1], f32)
            nc.vector.memset(nlnC[:], -float(np.log(C)))
            if 'E' in phases:
              with ExitStack() as es:
                sp = es.enter_context(tc.tile_pool(name="pE", bufs=3))
                pp = es.enter_context(tc.tile_pool(name="pEp", bufs=2, space="PSUM"))
                for m in range(NM):
                    hm = sp.tile([128, 2 * H], f32, tag="hm")
                    nc.sync.dma_start(
                        hm[:], hc1_d[:].rearrange("t b h -> (t b) h")[m * 128 : (m + 1) * 128, :])
                    ps = pp.tile([128, C], f32, tag="pse")
                    for k in range(4):
                        tps = pp.tile([128, 128], f32, tag="tp")
                        nc.tensor.transpose(out=tps[:], in_=hm[:, k * 128 : (k + 1) * 128], identity=ident[:])
                        ht = sp.tile([128, 128], f32, tag="ht")
                        nc.scalar.copy(out=ht[:], in_=tps[:])
                        nc.tensor.matmul(
                            ps[:], lhsT=ht[:], rhs=fct_sb[k][:],
                            start=(k == 0), stop=(k == 3))
                    em = sp.tile([128, C], f32, tag="em")
                    nc.vector.scalar_tensor_tensor(
                        out=em[:], in0=ps[:], scalar=0.0, in1=fcb_sb[:],
                        op0=OP.add, op1=OP.add)
                    junk = sp.tile([128, C], f32, tag="junk")
                    nc.vector.scalar_tensor_tensor(
                        out=junk[:], in0=em[:], scalar=0.0, in1=mask_sb[:, m * C : (m + 1) * C],
                        op0=OP.add, op1=OP.mult, accum_out=accT[:, m : m + 1])
                    ee = sp.tile([128, C], f32, tag="ee")
                    nc.scalar.activation(ee[:], em[:], AF.Exp, bias=nlnC[:, :1])
                    tps = pp.tile([128, 128], f32, tag="tp2")
                    nc.tensor.transpose(out=tps[:C, :], in_=ee[:], identity=ident[:])
                    nc.scalar.copy(out=ET[:, m * 128 : (m + 1) * 128], in_=tps[:C, :])

                # numE[b] = sum_p sel[p,b] * rowsum(accT)
                accR = sp.tile([128, 1], f32, tag="accR")
                nc.vector.tensor_reduce(accR[:], accT[:], axis=mybir.AxisListType.X, op=OP.add)
                psn = pp.tile([16, 1], f32, tag="psn")
                nc.tensor.matmul(psn[:], lhsT=sel_sb[:], rhs=accR[:], start=True, stop=True)
                neo = sp.tile([16, 1], f32, tag="neo")
                nc.scalar.copy(out=neo[:], in_=psn[:])
                nc.sync.dma_start(ne_out[:].rearrange("a b -> b a"), neo[:])

            # ---------------- Phase F: CRF forward algorithm (exp domain) ----------
            if 'F' not in phases:
                for out_ap in (s_out, ne_out, la_out):
                    pass
            if 'F' in phases:
              with ExitStack() as es:
                sp = es.enter_context(tc.tile_pool(name="pF", bufs=4))
                pp = es.enter_context(tc.tile_pool(name="pFp", bufs=2, space="PSUM"))
                logacc = cp.tile([1, B_LOC], f32)
                nc.vector.memset(logacc[:], 0.0)
                a = sp.tile([C, B_LOC], f32, tag="a0")
                nc.vector.tensor_scalar(a[:], ET[:, 0:B_LOC], est_sb[:, :1], None, op0=OP.mult)
                for t in range(1, T):
                    ps = pp.tile([C, B_LOC], f32, tag="psa")
                    nc.tensor.matmul(ps[:], lhsT=pm_sb[:], rhs=a[:], start=True, stop=True)
                    a = sp.tile([C, B_LOC], f32, tag=f"a{t % 3 + 1}")
                    nc.vector.scalar_tensor_tensor(
                        out=a[:], in0=ps[:], scalar=0.0,
                        in1=ET[:, t * B_LOC : (t + 1) * B_LOC],
                        op0=OP.add, op1=OP.mult)
                    if t % RESCALE_EVERY == 0:
                        nrm = pp.tile([C, B_LOC], f32, tag="nrm")
                        nc.tensor.matmul(nrm[:], lhsT=ones20[:], rhs=a[:], start=True, stop=True)
                        lnn = sp.tile([1, B_LOC], f32, tag="lnn")
                        nc.scalar.activation(lnn[:], nrm[:1, :], AF.Ln)
                        nc.vector.tensor_tensor(
                            out=logacc[:], in0=logacc[:], in1=lnn[:], op=OP.add)
                        rcp = sp.tile([C, B_LOC], f32, tag="rcp")
                        nc.vector.reciprocal(rcp[:], nrm[:])
                        a2 = sp.tile([C, B_LOC], f32, tag="a_rs")
                        nc.vector.tensor_tensor(out=a2[:], in0=a[:], in1=rcp[:], op=OP.mult)
                        a = a2
                af = sp.tile([C, B_LOC], f32, tag="af")
                nc.vector.tensor_scalar(af[:], a[:], een_sb[:, :1], None, op0=OP.mult)
                pss = pp.tile([1, B_LOC], f32, tag="pss")
                nc.tensor.matmul(pss[:], lhsT=ones20[:, :1], rhs=af[:], start=True, stop=True)
                so = sp.tile([1, B_LOC], f32, tag="so")
                nc.scalar.copy(out=so[:], in_=pss[:])
                nc.sync.dma_start(s_out[:], so[:])
                lao = sp.tile([1, B_LOC], f32, tag="lao")
                nc.vector.tensor_scalar(lao[:], logacc[:], chain_sb[:, :1], None, op0=OP.add)
                nc.sync.dma_start(la_out[:], lao[:])

        for _rep in range(reps):
            whole_model()

    nc.compile()
    return nc


def _prep_host(inputs, T):
    """Host-side weight transforms + per-core in_maps."""
    f32 = np.float32
    ids_full = np.asarray(inputs["input_ids"]).astype(np.int32)      # [B, T]
    labels = np.asarray(inputs["labels"]).astype(np.int64)           # [B, T]
    emb = np.asarray(inputs["emb"], dtype=f32)
    trans = np.asarray(inputs["transitions"], dtype=f32)
    start = np.asarray(inputs["start_trans"], dtype=f32)
    end = np.asarray(inputs["end_trans"], dtype=f32)

    colscale = np.ones(G4, f32)
    colscale[0:256] = 0.5       # i
    colscale[256:512] = 0.5     # f
    colscale[768:1024] = 0.5    # o

    def prep_layer(wi, wh, bi, bh, in_scale):
        # wi [2, 4H, in], wh [2, 4H, H] -> transposed, scaled
        wiT = np.ascontiguousarray(np.transpose(wi, (0, 2, 1))).astype(f32)
        whT = np.ascontiguousarray(np.transpose(wh, (0, 2, 1))).astype(f32)
        wiT = wiT * in_scale * colscale[None, None, :]
        whT = whT * 0.5 * colscale[None, None, :]
        b = (np.asarray(bi, f32) + np.asarray(bh, f32)) * colscale[None, :]
        return wiT, whT, b

    w0i, w0h, b0 = prep_layer(inputs["w_ih_l0"], inputs["w_hh_l0"],
                              inputs["b_ih_l0"], inputs["b_hh_l0"], 1.0)
    w1i, w1h, b1 = prep_layer(inputs["w_ih_l1"], inputs["w_hh_l1"],
                              inputs["b_ih_l1"], inputs["b_hh_l1"], 0.5)
    fcT = (np.asarray(inputs["fc_w"], f32).T * 0.5).astype(f32)      # [2H, C]
    fcb = np.asarray(inputs["fc_b"], f32)

    b0r = np.broadcast_to(b0[None, :, :], (128, 2, G4)).copy()
    b1r = np.broadcast_to(b1[None, :, :], (128, 2, G4)).copy()
    fcbr = np.broadcast_to(fcb[None, :], (128, C)).copy()
    Pm = np.exp(trans).astype(f32)
    est = np.exp(start).astype(f32).reshape(C, 1)
    een = np.exp(end).astype(f32).reshape(C, 1)
    selm = (np.arange(128)[:, None] % B_LOC == np.arange(B_LOC)[None, :]).astype(f32)

    NTOK = T * B_LOC
    NM = NTOK // 128
    in_maps = []
    host_num = np.zeros(B, np.float64)
    for c in range(N_CORES):
        bs = slice(c * B_LOC, (c + 1) * B_LOC)
        ids_c = ids_full[bs, :T].T.reshape(NTOK)                     # (t,b) t-major
        ids_tile = ids_c.reshape(NM, 128).T.copy()                   # [128, NM]
        lab_c = labels[bs, :T].T.reshape(NTOK)                       # token (t,b)
        maskE = np.zeros((128, NM * C), f32)
        toks = np.arange(NTOK)
        maskE[toks % 128, (toks // 128) * C + lab_c] = 1.0
        in_maps.append({
            "ids": ids_tile.astype(np.int32), "emb": emb,
            "w0i": w0i, "w0h": w0h, "b0r": b0r,
            "w1i": w1i, "w1h": w1h, "b1r": b1r,
            "fcT": fcT, "fcbr": fcbr, "maskE": maskE, "selm": selm,
            "Pm": Pm, "estart": est, "eend": een,
            "chain": np.zeros((1, 1), f32),
        })
        # host part of gold-path score (pure input data)
        lb = labels[bs, :T]
        host_num[c * B_LOC:(c + 1) * B_LOC] = (
            start[lb[:, 0]].astype(np.float64)
            + trans[lb[:, :-1], lb[:, 1:]].sum(-1)
            + end[lb[:, -1]]
        )
    return in_maps, host_num


def _run(inputs, T):
    from concourse.bass_utils import run_bass_kernel_spmd

    if T not in _COMPILED:
        _COMPILED[T] = _build(T)
    nc = _COMPILED[T]
    in_maps, host_num = _prep_host(inputs, T)
    res = run_bass_kernel_spmd(nc, in_maps, core_ids=list(range(N_CORES)))
    total = 0.0
    for c in range(N_CORES):
        r = res.results[c]
        S = r["S_out"].reshape(B_LOC).astype(np.float64)
        numE = r["numE_out"].reshape(B_LOC).astype(np.float64)
        logacc = r["logacc_out"].reshape(B_LOC).astype(np.float64)
        logZ = np.log(S) + logacc + T * np.log(C)
        num = host_num[c * B_LOC:(c + 1) * B_LOC] + numE
        total += (logZ - num).sum()
    return np.float32(total)


def kernel(**inputs):
    return _run(inputs, T_FULL)



# revision 7
# speedup vs baseline: 1.7431x; 1.7431x over previous
"""BiLSTM-CRF sequence tagging loss on 8 Trainium2 NeuronCores.

Data-parallel: batch 128 sharded 16/core across 8 cores; each core runs the
full model (embedding gather, 2 BiLSTM layers, FC, CRF forward algorithm)
on its own shard with zero cross-core communication. Host sums the 8
per-core partial losses.

v2 redesign vs the first working kernel:
  - Transposed-state recurrence: hidden/cell state kept as h^T tiles
    [128 gate-rows, 16 batch] so the recurrent matmul needs NO per-step
    transposes and all elementwise work runs on 128 partitions.
  - Gate preactivations are accumulated entirely in PSUM: per 8-step block,
    a bias-broadcast matmul (start=True) then the input-projection matmuls
    and finally the per-step W_hh matmuls (all start=False) add into the
    same PSUM region; tanh reads PSUM directly. No per-step DMA, no
    gate-add on DVE.
  - All large matmuls in bf16 (fp32 PE matmul streams at 1/4 rate).
  - sigmoid-free LSTM cell as before: i,f,o columns pre-scaled by 0.5 so a
    single tanh covers all four gates, with doubled cell/hidden state.
  - Embeddings gathered+transposed once into SBUF (bf16); hcat layers
    round-trip DRAM in the transposed layout (bf16) so layer 1 / FC consume
    them directly as matmul operands.
  - CRF partition function in the exp domain as before; the per-rescale Ln
    calls are batched into one Ln at the end (no ACT table thrash).
"""

import numpy as np

V, E, H, C = 50000, 300, 256, 20
B, T_FULL = 128, 512
N_CORES = 8
B_LOC = B // N_CORES  # 16
G4 = 4 * H  # 1024
TB = 8                 # time steps per block
RESCALE_EVERY = 24

_COMPILED = {}


def _build(T, debug=False, phases='ABCDEF', reps=1):
    import concourse.bass as bass
    import concourse.mybir as mybir
    import concourse.tile as tile
    from concourse import bacc
    from concourse.masks import make_identity
    from contextlib import ExitStack

    f32 = mybir.dt.float32
    bf16 = mybir.dt.bfloat16
    i32 = mybir.dt.int32
    AF = mybir.ActivationFunctionType
    OP = mybir.AluOpType

    NTOK = T * B_LOC            # tokens per core (t-major: tok = t*16 + b)
    NBLK = T // TB              # 8-step blocks
    E_CH = [(0, 128), (128, 128), (256, 44)]
    NRESC = (T - 1) // RESCALE_EVERY  # number of rescales in phase F

    nc = bacc.Bacc("TRN2", debug=False, num_devices=N_CORES)

    def din(name, shape, dt=f32):
        return nc.dram_tensor(name, shape, dt, kind="ExternalInput").ap()

    ids_d = din("ids", (128, NBLK), i32)
    emb_d = din("emb", (V, E))
    w0i_d = din("w0i", (2, E, G4), bf16)
    w0h_d = din("w0h", (2, H, G4), bf16)
    b0g_d = din("b0g", (2, 8, 128), bf16)
    w1i_d = din("w1i", (2, 2 * H, G4), bf16)
    w1h_d = din("w1h", (2, H, G4), bf16)
    b1g_d = din("b1g", (2, 8, 128), bf16)
    ind4_d = din("ind4", (4, 512), bf16)
    fct_d = din("fcT", (2 * H, C), bf16)
    fcb_d = din("fcb", (1, C), bf16)
    ones_d = din("ones1", (1, 128), bf16)
    mask_d = din("maskT", (C, NTOK), bf16)
    pm_d = din("Pm", (C, C))
    est_d = din("estart", (C, 1))
    een_d = din("eend", (C, 1))

    s_out = nc.dram_tensor("S_out", (1, B_LOC), f32, kind="ExternalOutput").ap()
    ne_out = nc.dram_tensor("numE_out", (1, B_LOC), f32, kind="ExternalOutput").ap()
    la_out = nc.dram_tensor("logacc_out", (1, B_LOC), f32, kind="ExternalOutput").ap()

    # DRAM scratch: transposed hidden history [dir, k-chunk, 128, T*16] bf16
    dbg_kind = "ExternalOutput" if debug else "Internal"
    hc0_d = nc.dram_tensor("hc0", (2, 2, 128, NTOK), bf16, kind=dbg_kind).ap()
    hc1_d = nc.dram_tensor("hc1", (2, 2, 128, NTOK), bf16, kind=dbg_kind).ap()

    with tile.TileContext(nc) as tc, ExitStack() as top:
        cp = top.enter_context(tc.tile_pool(name="const", bufs=1))
        # shared PSUM pools: exactly 8 banks total
        pg = top.enter_context(tc.tile_pool(name="pgate", bufs=1, space="PSUM"))
        paux = top.enter_context(tc.tile_pool(name="paux", bufs=2, space="PSUM"))
        pf = top.enter_context(tc.tile_pool(name="pF", bufs=2, space="PSUM"))

        ident = cp.tile([128, 128], f32)
        make_identity(nc, ident[:])

        ids_sb = cp.tile([128, NBLK], i32)
        nc.sync.dma_start(ids_sb[:], ids_d[:])

        def load_const(name, dram, shape, dt=f32):
            t = cp.tile(list(shape), dt, tag=name, name=name)
            nc.sync.dma_start(t[:], dram)
            return t

        # weights, transposed layouts, bf16
        w0i_sb = [[load_const(f"w0i{d}_{ki}", w0i_d[d, r0:r0 + ck, :], (ck, G4), bf16)
                   for ki, (r0, ck) in enumerate(E_CH)] for d in range(2)]
        w0h_sb = [[load_const(f"w0h{d}_{k}", w0h_d[d, k * 128:(k + 1) * 128, :], (128, G4), bf16)
                   for k in range(2)] for d in range(2)]
        w1i_sb = [[load_const(f"w1i{d}_{k}", w1i_d[d, k * 128:(k + 1) * 128, :], (128, G4), bf16)
                   for k in range(4)] for d in range(2)]
        w1h_sb = [[load_const(f"w1h{d}_{k}", w1h_d[d, k * 128:(k + 1) * 128, :], (128, G4), bf16)
                   for k in range(2)] for d in range(2)]
        b0g_sb = [[load_const(f"b0g{d}_{jg}", b0g_d[d, jg * 4:(jg + 1) * 4, :], (4, 128), bf16)
                   for jg in range(2)] for d in range(2)]
        b1g_sb = [[load_const(f"b1g{d}_{jg}", b1g_d[d, jg * 4:(jg + 1) * 4, :], (4, 128), bf16)
                   for jg in range(2)] for d in range(2)]
        ind4_sb = load_const("ind4", ind4_d[:], (4, 512), bf16)
        fct_sb = [load_const(f"fct{k}", fct_d[k * 128:(k + 1) * 128, :], (128, C), bf16)
                  for k in range(4)]
        fcb_sb = load_const("fcb", fcb_d[:], (1, C), bf16)
        ones_sb = load_const("ones1", ones_d[:], (1, 128), bf16)
        mask_sb = load_const("maskT", mask_d[:], (C, NTOK), bf16)
        pm_sb = load_const("Pm", pm_d[:], (C, C))
        est_sb = load_const("est", est_d[:], (C, 1))
        een_sb = load_const("een", een_d[:], (C, 1))
        ones20 = cp.tile([C, C], f32)
        nc.vector.memset(ones20[:], 1.0)
        nlnC = cp.tile([C, 1], f32)
        nc.vector.memset(nlnC[:], -float(np.log(C)))

        # embeddings, gathered + transposed once: xT[k] = [128, NTOK] bf16
        xT = [cp.tile([128, NTOK], bf16, tag=f"xT{k}", name=f"xT{k}") for k in range(3)]

        # persistent phase E/F tiles
        ET = cp.tile([C, NTOK], f32)
        numacc = cp.tile([C, NTOK], f32)
        stash = cp.tile([1, max(NRESC, 1) * B_LOC], f32)
        logacc = cp.tile([1, B_LOC], f32)

        def gather_embeddings():
            with ExitStack() as es:
                sp = es.enter_context(tc.tile_pool(name="pGa", bufs=3))
                for m in range(NBLK):
                    xm = sp.tile([128, E], f32, tag="xm")
                    nc.gpsimd.indirect_dma_start(
                        out=xm[:], out_offset=None, in_=emb_d[:],
                        in_offset=bass.IndirectOffsetOnAxis(ap=ids_sb[:, m:m + 1], axis=0),
                    )
                    for k, (r0, ck) in enumerate(E_CH):
                        tp = paux.tile([128, 128], f32, tag="aux", name=f"tp{m}_{k}")
                        nc.tensor.transpose(out=tp[:ck, :], in_=xm[:, r0:r0 + ck], identity=ident[:])
                        nc.vector.tensor_copy(xT[k][:ck, m * 128:(m + 1) * 128], tp[:ck, :])

        def recurrence(layer):
            """One BiLSTM layer, both directions interleaved, 8-step blocks."""
            wh_sb = w0h_sb if layer == 0 else w1h_sb
            wi_sb = w0i_sb if layer == 0 else w1i_sb
            bg_sb = b0g_sb if layer == 0 else b1g_sb
            hout_d = hc0_d if layer == 0 else hc1_d

            with ExitStack() as es:
                sp = es.enter_context(tc.tile_pool(name=f"rec{layer}", bufs=3))
                hp = es.enter_context(tc.tile_pool(name=f"rech{layer}", bufs=2))

                cT = [cp.tile([128, 32], f32, tag=f"cT{layer}{d}", name=f"cT{layer}{d}") for d in range(2)]
                for d in range(2):
                    nc.vector.memset(cT[d][:], 0.0)

                prev_h = [None, None]   # previous block's hstg tile per dir

                def xg_block(d, blk, pxg):
                    """Bias + input projection for tokens of `blk` into PSUM."""
                    c0 = blk * 128
                    for jg in range(2):
                        nc.tensor.matmul(
                            pxg[:, jg * 512:(jg + 1) * 512],
                            lhsT=bg_sb[d][jg][:],
                            rhs=ind4_sb[:],
                            start=True, stop=False, skip_group_check=True)
                    if layer == 0:
                        for j in range(8):
                            for k, (r0, ck) in enumerate(E_CH):
                                nc.tensor.matmul(
                                    pxg[:, j * 128:(j + 1) * 128],
                                    lhsT=wi_sb[d][k][:ck, j * 128:(j + 1) * 128],
                                    rhs=xT[k][:ck, c0:c0 + 128],
                                    start=False, stop=False, skip_group_check=True)
                    else:
                        hcin = sp.tile([128, 512], bf16, tag=f"hcin{d}", name=f"hcin{d}_{blk}")
                        nc.sync.dma_start(
                            hcin[:], hc0_d[:, :, :, c0:c0 + 128].rearrange("a k p c -> p a k c"))
                        for j in range(8):
                            for k in range(4):
                                nc.tensor.matmul(
                                    pxg[:, j * 128:(j + 1) * 128],
                                    lhsT=wi_sb[d][k][:, j * 128:(j + 1) * 128],
                                    rhs=hcin[:, k * 128:(k + 1) * 128],
                                    start=False, stop=False, skip_group_check=True)

                def step(d, t, first, pxg, hstg):
                    """One recurrence step at global time t (slot t%TB)."""
                    sl = t % TB
                    if not first:
                        # rhs = h(prev step) from current/previous staging tile
                        pt = t - 1 if d == 0 else t + 1
                        src = hstg if (pt // TB) == (t // TB) else prev_h[d]
                        psl = pt % TB
                        for j in range(8):
                            for k in range(2):
                                nc.tensor.matmul(
                                    pxg[:, j * 128 + sl * 16: j * 128 + sl * 16 + 16],
                                    lhsT=wh_sb[d][k][:, j * 128:(j + 1) * 128],
                                    rhs=src[:, k * 128 + psl * 16: k * 128 + psl * 16 + 16],
                                    start=False, stop=(k == 1), skip_group_check=True)
                    gv = pxg[:].rearrange("p (j t b) -> p j t b", j=8, t=TB, b=16)[:, :, sl, :]
                    Tall = sp.tile([128, 128], f32, tag=f"Tall{d}", name=f"Tall{d}_{t}")
                    nc.scalar.activation(Tall[:].rearrange("p (j b) -> p j b", j=8, b=16), gv, AF.Tanh)
                    A = sp.tile([128, 32], f32, tag=f"A{d}", name=f"A{d}_{t}")
                    nc.vector.scalar_tensor_tensor(
                        out=A[:], in0=Tall[:, 32:64], scalar=1.0, in1=cT[d][:],
                        op0=OP.add, op1=OP.mult)
                    Bv = sp.tile([128, 32], f32, tag=f"Bv{d}", name=f"Bv{d}_{t}")
                    nc.vector.scalar_tensor_tensor(
                        out=Bv[:], in0=Tall[:, 0:32], scalar=1.0, in1=Tall[:, 64:96],
                        op0=OP.add, op1=OP.mult)
                    nc.vector.scalar_tensor_tensor(
                        out=cT[d][:], in0=A[:], scalar=0.5, in1=Bv[:],
                        op0=OP.mult, op1=OP.add)
                    TC = sp.tile([128, 32], f32, tag=f"TC{d}", name=f"TC{d}_{t}")
                    nc.scalar.activation(TC[:], cT[d][:], AF.Tanh, scale=0.5)
                    hv = hstg[:].rearrange("p (k t b) -> p k t b", k=2, t=TB, b=16)[:, :, sl, :]
                    nc.vector.scalar_tensor_tensor(
                        out=hv, in0=Tall[:, 96:128], scalar=1.0,
                        in1=TC[:].rearrange("p (k b) -> p k b", k=2, b=16),
                        op0=OP.add, op1=OP.mult)

                for bi in range(NBLK):
                    blk = [bi, NBLK - 1 - bi]   # fwd ascending, bwd descending
                    pxg = [pg.tile([128, 8 * 128], f32, tag=f"pxg{d}", name=f"pxg{d}_{bi}")
                           for d in range(2)]
                    hstg = [hp.tile([128, 2 * TB * 16], bf16, tag=f"hstg{d}", name=f"hstg{d}_{bi}")
                            for d in range(2)]
                    for d in range(2):
                        xg_block(d, blk[d], pxg[d])
                    for i in range(TB):
                        for d in range(2):
                            t = blk[d] * TB + (i if d == 0 else TB - 1 - i)
                            first = (bi == 0 and i == 0)
                            step(d, t, first, pxg[d], hstg[d])
                    for d in range(2):
                        c0 = blk[d] * 128
                        nc.sync.dma_start(
                            hout_d[d, :, :, c0:c0 + 128].rearrange("k p c -> p k c"),
                            hstg[d][:])
                        prev_h[d] = hstg[d]

        def emissions():
            with ExitStack() as es:
                sp = es.enter_context(tc.tile_pool(name="pE", bufs=3))
                for m in range(NBLK):
                    c0 = m * 128
                    hcin = sp.tile([128, 512], bf16, tag="hcin1", name=f"hc1_{m}")
                    nc.sync.dma_start(
                        hcin[:], hc1_d[:, :, :, c0:c0 + 128].rearrange("a k p c -> p a k c"))
                    ps = paux.tile([128, 128], f32, tag="aux", name=f"emT{m}")
                    nc.tensor.matmul(ps[:C, :], lhsT=fcb_sb[:], rhs=ones_sb[:],
                                     start=True, stop=False, skip_group_check=True)
                    for k in range(4):
                        nc.tensor.matmul(
                            ps[:C, :], lhsT=fct_sb[k][:], rhs=hcin[:, k * 128:(k + 1) * 128],
                            start=False, stop=(k == 3), skip_group_check=True)
                    # numerator: emissions at the gold labels, via mask multiply
                    nc.vector.scalar_tensor_tensor(
                        out=numacc[:, c0:c0 + 128], in0=ps[:C, :], scalar=0.0,
                        in1=mask_sb[:, c0:c0 + 128], op0=OP.add, op1=OP.mult)
                    # exp(e - ln C) transposed for the CRF
                    nc.scalar.activation(ET[:, c0:c0 + 128], ps[:C, :], AF.Exp,
                                         bias=nlnC[:, :1])
                # numE[b] = sum_c sum_t numacc[c, t*16+b]
                nred = sp.tile([C, B_LOC], f32, tag="nred")
                nc.vector.tensor_reduce(
                    nred[:].rearrange("p (b o) -> p b o", o=1),
                    numacc[:].rearrange("p (t b) -> p b t", t=T, b=B_LOC),
                    axis=mybir.AxisListType.X, op=OP.add)
                psn = pf.tile([1, B_LOC], f32, tag="fps", name="psnum")
                nc.tensor.matmul(psn[:], lhsT=ones20[:, :1], rhs=nred[:],
                                 start=True, stop=True)
                neo = sp.tile([1, B_LOC], f32, tag="neo")
                nc.scalar.copy(out=neo[:], in_=psn[:])
                nc.sync.dma_start(ne_out[:], neo[:])

        def crf():
            with ExitStack() as es:
                sp = es.enter_context(tc.tile_pool(name="pCRF", bufs=4))
                nc.vector.memset(logacc[:], 0.0)
                a = sp.tile([C, B_LOC], f32, tag="a0")
                nc.vector.tensor_scalar(a[:], ET[:, 0:B_LOC], est_sb[:, :1], None, op0=OP.mult)
                nresc = 0
                for t in range(1, T):
                    ps = pf.tile([C, B_LOC], f32, tag="fps", name=f"psa{t}")
                    nc.tensor.matmul(ps[:], lhsT=pm_sb[:], rhs=a[:], start=True, stop=True)
                    a = sp.tile([C, B_LOC], f32, tag=f"a{t % 3 + 1}")
                    nc.vector.scalar_tensor_tensor(
                        out=a[:], in0=ps[:], scalar=0.0,
                        in1=ET[:, t * B_LOC:(t + 1) * B_LOC],
                        op0=OP.add, op1=OP.mult)
                    if t % RESCALE_EVERY == 0:
                        nrm = pf.tile([C, B_LOC], f32, tag="fps", name=f"nrm{t}")
                        nc.tensor.matmul(nrm[:], lhsT=ones20[:], rhs=a[:], start=True, stop=True)
                        nc.scalar.copy(out=stash[:, (nresc) * B_LOC:(nresc + 1) * B_LOC],
                                       in_=nrm[:1, :])
                        nresc += 1
                        rcp = sp.tile([C, B_LOC], f32, tag="rcp")
                        nc.vector.reciprocal(rcp[:], nrm[:])
                        a2 = sp.tile([C, B_LOC], f32, tag="a_rs")
                        nc.vector.tensor_tensor(out=a2[:], in0=a[:], in1=rcp[:], op=OP.mult)
                        a = a2
                af = sp.tile([C, B_LOC], f32, tag="af")
                nc.vector.tensor_scalar(af[:], a[:], een_sb[:, :1], None, op0=OP.mult)
                pss = pf.tile([1, B_LOC], f32, tag="fps", name="pss")
                nc.tensor.matmul(pss[:], lhsT=ones20[:, :1], rhs=af[:], start=True, stop=True)
                so = sp.tile([1, B_LOC], f32, tag="so")
                nc.scalar.copy(out=so[:], in_=pss[:])
                nc.sync.dma_start(s_out[:], so[:])
                # logacc[b] = sum_r ln(stash[r, b]) -- one batched Ln
                lns = sp.tile([1, NRESC * B_LOC], f32, tag="lns")
                nc.scalar.activation(lns[:], stash[:, :NRESC * B_LOC], AF.Ln)
                nc.vector.tensor_reduce(
                    logacc[:].rearrange("p (b o) -> p b o", o=1),
                    lns[:].rearrange("p (r b) -> p b r", r=NRESC, b=B_LOC),
                    axis=mybir.AxisListType.X, op=OP.add)
                nc.sync.dma_start(la_out[:], logacc[:])

        def whole_model():
            if 'A' in phases:
                gather_embeddings()
            if 'B' in phases:
                recurrence(0)
            if 'D' in phases:
                recurrence(1)
            if 'E' in phases:
                emissions()
            if 'F' in phases:
                crf()

        for _rep in range(reps):
            whole_model()

    nc.compile()
    return nc


def _prep_host(inputs, T):
    """Host-side weight transforms + per-core in_maps."""
    import ml_dtypes
    f32 = np.float32
    bf16 = ml_dtypes.bfloat16
    NTOK = T * B_LOC
    NBLK = T // TB
    ids_full = np.asarray(inputs["input_ids"]).astype(np.int32)      # [B, T]
    labels = np.asarray(inputs["labels"]).astype(np.int64)           # [B, T]
    emb = np.asarray(inputs["emb"], dtype=f32)
    trans = np.asarray(inputs["transitions"], dtype=f32)
    start = np.asarray(inputs["start_trans"], dtype=f32)
    end = np.asarray(inputs["end_trans"], dtype=f32)

    colscale = np.ones(G4, f32)
    colscale[0:256] = 0.5       # i
    colscale[256:512] = 0.5     # f
    colscale[768:1024] = 0.5    # o

    def prep_layer(wi, wh, bi, bh, in_scale):
        wiT = np.ascontiguousarray(np.transpose(wi, (0, 2, 1))).astype(f32)
        whT = np.ascontiguousarray(np.transpose(wh, (0, 2, 1))).astype(f32)
        wiT = wiT * in_scale * colscale[None, None, :]
        whT = whT * 0.5 * colscale[None, None, :]
        b = (np.asarray(bi, f32) + np.asarray(bh, f32)) * colscale[None, :]
        return wiT.astype(bf16), whT.astype(bf16), b.reshape(2, 8, 128).astype(bf16)

    w0i, w0h, b0g = prep_layer(inputs["w_ih_l0"], inputs["w_hh_l0"],
                               inputs["b_ih_l0"], inputs["b_hh_l0"], 1.0)
    w1i, w1h, b1g = prep_layer(inputs["w_ih_l1"], inputs["w_hh_l1"],
                               inputs["b_ih_l1"], inputs["b_hh_l1"], 0.5)
    fcT = (np.asarray(inputs["fc_w"], f32).T * 0.5).astype(bf16)     # [2H, C]
    fcb = np.asarray(inputs["fc_b"], f32).reshape(1, C).astype(bf16)
    ind4 = (np.arange(512)[None, :] // 128 == np.arange(4)[:, None]).astype(bf16)
    ones1 = np.ones((1, 128), bf16)
    Pm = np.exp(trans).astype(f32)
    est = np.exp(start).astype(f32).reshape(C, 1)
    een = np.exp(end).astype(f32).reshape(C, 1)

    in_maps = []
    host_num = np.zeros(B, np.float64)
    for c in range(N_CORES):
        bs = slice(c * B_LOC, (c + 1) * B_LOC)
        ids_c = ids_full[bs, :T].T.reshape(NTOK)                     # (t,b) t-major
        ids_tile = ids_c.reshape(NBLK, 128).T.copy()                 # [128, NBLK]
        lab_c = labels[bs, :T].T.reshape(NTOK)                       # token (t,b)
        maskT = np.zeros((C, NTOK), f32)
        maskT[lab_c, np.arange(NTOK)] = 1.0
        in_maps.append({
            "ids": ids_tile.astype(np.int32), "emb": emb,
            "w0i": w0i, "w0h": w0h, "b0g": b0g,
            "w1i": w1i, "w1h": w1h, "b1g": b1g,
            "ind4": ind4, "fcT": fcT, "fcb": fcb, "ones1": ones1,
            "maskT": maskT.astype(bf16),
            "Pm": Pm, "estart": est, "eend": een,
        })
        lb = labels[bs, :T]
        host_num[c * B_LOC:(c + 1) * B_LOC] = (
            start[lb[:, 0]].astype(np.float64)
            + trans[lb[:, :-1], lb[:, 1:]].sum(-1)
            + end[lb[:, -1]]
        )
    return in_maps, host_num


def _run(inputs, T):
    from concourse.bass_utils import run_bass_kernel_spmd

    if T not in _COMPILED:
        _COMPILED[T] = _build(T)
    nc = _COMPILED[T]
    in_maps, host_num = _prep_host(inputs, T)
    res = run_bass_kernel_spmd(nc, in_maps, core_ids=list(range(N_CORES)))
    total = 0.0
    for c in range(N_CORES):
        r = res.results[c]
        S = r["S_out"].reshape(B_LOC).astype(np.float64)
        numE = r["numE_out"].reshape(B_LOC).astype(np.float64)
        logacc = r["logacc_out"].reshape(B_LOC).astype(np.float64)
        logZ = np.log(S) + logacc + T * np.log(C)
        num = host_num[c * B_LOC:(c + 1) * B_LOC] + numE
        total += (logZ - num).sum()
    return np.float32(total)


def kernel(**inputs):
    return _run(inputs, T_FULL)
